# revision 9
# baseline (speedup 1.0000x reference)
"""BiDAF attention (nn_BertBidafAttention) on 8 TRN2 NeuronCores.

Math (per batch, reference):
    cp = c @ W.T + b            [CL, H]
    s  = cp @ q.T               [CL, QL]
    s1 = softmax_q(s + qmask_bias)      (row softmax)
    s2 = softmax_c(s + cmask_bias)      (col softmax)
    a  = s1 @ q                 [CL, H]
    bv = (s1 @ s2.T) @ c        [CL, H]
    x  = [c, a, c*a, c*bv]      [CL, 4H]

Restructured for the PE:
    qW[q,h] = sum_d q[q,d] W[d,h]       (75 MF instead of 604 MF for cp)
    s[c,q]  = sum_h c[c,h] qW[q,h] + qb[q],  qb = q @ b
    bv      = s1 @ (s2.T @ c)           (avoids the [CL,CL] product)
Mask biases (-30000 on masked entries) and qb are folded into the PSUM
accumulation as rank-1 (K=1) matmuls, so softmax is plain max/exp/sum.
qb cancels inside the column softmax s2 and is omitted there.

Sharding: data-parallel over batch, 2 batches per core, no collectives.
"""

import numpy as np
from contextlib import ExitStack

import concourse.bass as bass
from concourse import bacc
import concourse.mybir as mybir
import concourse.tile as tile
from concourse.masks import make_identity
from concourse.bass_utils import run_bass_kernel_spmd

B, CL, QL, H = 16, 512, 64, 768
NCORES = 8
BPC = B // NCORES  # batches per core
HK = H // 128      # 6 k-tiles over the feature dims
CT = CL // 128     # 4 c-tiles
NH = H // 2        # 384, N per matmul half (<=512 fp32 moving limit)
NEGB = -30000.0    # additive mask bias; exp(NEGB - max) == 0.0 in fp32

f32 = mybir.dt.float32
f32r = mybir.dt.float32r
i32 = mybir.dt.int32


def _build_nc(use_f32r: bool = True) -> bass.Bass:
    nc = bacc.Bacc()
    cD = nc.declare_dram_parameter("c", [BPC, CL, H], f32, isOutput=False)
    qD = nc.declare_dram_parameter("q", [BPC, QL, H], f32, isOutput=False)
    cmD = nc.declare_dram_parameter("c_mask", [BPC, CL], i32, isOutput=False)
    qmD = nc.declare_dram_parameter("q_mask", [BPC, QL], i32, isOutput=False)
    WD = nc.declare_dram_parameter("W", [H, H], f32, isOutput=False)
    bD = nc.declare_dram_parameter("b", [H], f32, isOutput=False)
    outD = nc.declare_dram_parameter("out", [BPC, CL, 4 * H], f32, isOutput=True)

    # float32r runs the PE at full rate for free-dim >= 256 (vs 4 cycles/row
    # for plain fp32). The BIR verifier requires every f32r matmul operand to
    # be *written* as f32r (rounded by its producer), so the post-softmax
    # value tensors (s1T, qc, q copy) get dedicated f32r-typed tiles; the
    # logits stay full fp32.
    rdt = f32r if use_f32r else f32

    with tile.TileContext(nc) as tc, ExitStack() as ctx:
        const = ctx.enter_context(tc.tile_pool(name="const", bufs=1))
        wpool = ctx.enter_context(tc.tile_pool(name="wpool", bufs=1))
        perb = ctx.enter_context(tc.tile_pool(name="perb", bufs=2))
        small = ctx.enter_context(tc.tile_pool(name="small", bufs=2))
        outp = ctx.enter_context(tc.tile_pool(name="outp", bufs=3))
        ptp = ctx.enter_context(tc.tile_pool(name="ptp", bufs=2, space="PSUM"))
        pacc = ctx.enter_context(tc.tile_pool(name="pacc", bufs=4, space="PSUM"))

        ident = const.tile([128, 128], f32)
        make_identity(nc, ident)
        ones = const.tile([1, 128], f32)
        nc.vector.memset(ones, 1.0)

        # --- shared weights ---
        w_sb = wpool.tile([128, HK, H], f32)
        nc.sync.dma_start(out=w_sb, in_=WD[:].rearrange("(k p) h -> p k h", p=128))
        b_sb = wpool.tile([128, HK], f32)
        nc.sync.dma_start(out=b_sb, in_=bD[:].rearrange("(k p) -> p k", p=128))

        # --- mask bias rows (computed once for both batches) ---
        # int32 -> fp32 cast during SWDGE DMA.
        qmf = small.tile([1, BPC, QL], f32, tag="qmf", bufs=1)
        nc.gpsimd.dma_start(out=qmf[:1].rearrange("o b l -> o (b l)"),
                            in_=qmD[:].rearrange("b (o l) -> o (b l)", o=1))
        cmf = small.tile([1, BPC, CL], f32, tag="cmf", bufs=1)
        nc.gpsimd.dma_start(out=cmf[:1].rearrange("o b l -> o (b l)"),
                            in_=cmD[:].rearrange("b (o l) -> o (b l)", o=1))
        # bias = (mask - 1) * |NEGB|  ->  0 where mask==1, NEGB where mask==0
        qbias = small.tile([1, BPC, QL], f32, tag="qbias", bufs=1)
        nc.scalar.activation(qbias, qmf, mybir.ActivationFunctionType.Copy,
                             bias=NEGB, scale=-NEGB)
        cbias = small.tile([1, BPC, CL], f32, tag="cbias", bufs=1)
        nc.scalar.activation(cbias, cmf, mybir.ActivationFunctionType.Copy,
                             bias=NEGB, scale=-NEGB)

        # --- load q, transpose to qT [d, (b q)] ---
        q_nat = []
        qT2 = wpool.tile([128, HK, BPC, QL], f32)  # [d, k, b, q]
        q_r = []
        for bi in range(BPC):
            qn = perb.tile([64, H], f32, tag="q_nat")
            nc.sync.dma_start(out=qn, in_=qD[bi])
            q_nat.append(qn)
            qr = perb.tile([64, H], rdt, tag="q_r")
            nc.sync.dma_start(out=qr, in_=qD[bi].bitcast(rdt))
            q_r.append(qr)
            for k in range(HK):
                tp = ptp.tile([128, QL], f32, tag="tp")
                nc.tensor.transpose(tp, qn[:, k * 128:(k + 1) * 128],
                                    ident[:64, :64])
                nc.scalar.copy(out=qT2[:, k, bi, :], in_=tp)

        # --- qWT[h, (b q)] = sum_d W[d,h] qT[d, (b q)] ---
        qwt = wpool.tile([128, HK, BPC * QL], f32)  # [h, hm, (b q)]
        for hm in range(HK):
            ps = pacc.tile([128, BPC * QL], f32, tag="acc")
            for k in range(HK):
                nc.tensor.matmul(ps, w_sb[:, k, hm * 128:(hm + 1) * 128],
                                 qT2[:, k].rearrange("p b q -> p (b q)"),
                                 start=(k == 0), stop=(k == HK - 1))
            nc.scalar.copy(out=qwt[:, hm, :], in_=ps)

        # --- qb[(b q)] = sum_d b[d] qT[d, (b q)] ---
        ps_qb = pacc.tile([1, BPC * QL], f32, tag="acc")
        for k in range(HK):
            nc.tensor.matmul(ps_qb, b_sb[:, k:k + 1],
                             qT2[:, k].rearrange("p b q -> p (b q)"),
                             start=(k == 0), stop=(k == HK - 1))
        # row bias for the [c,q]-layout logits: qb + qmask_bias
        qrow = small.tile([1, BPC, QL], f32, tag="qrow", bufs=1)
        nc.vector.tensor_add(qrow[:1].rearrange("o b l -> o (b l)"),
                             ps_qb[:1, :],
                             qbias[:1].rearrange("o b l -> o (b l)"))

        for bi in range(BPC):
            # --- load c, build cT via PE transposes ---
            c_nat = perb.tile([128, CT, H], f32, tag="c_nat")
            nc.sync.dma_start(out=c_nat,
                              in_=cD[bi].rearrange("(t p) h -> p t h", p=128))
            cT = perb.tile([128, HK, CL], f32, tag="cT")
            for ci in range(CT):
                for k in range(HK):
                    tp = ptp.tile([128, 128], f32, tag="tp")
                    nc.tensor.transpose(tp, c_nat[:, ci, k * 128:(k + 1) * 128],
                                        ident)
                    nc.vector.tensor_copy(out=cT[:, k, ci * 128:(ci + 1) * 128],
                                          in_=tp)

            # --- sT[q, c] (for the column softmax s2); qb cancels here ---
            ps_st = pacc.tile([64, CL], f32, tag="acc")
            for k in range(HK):
                nc.tensor.matmul(ps_st, qwt[:, k, bi * QL:(bi + 1) * QL],
                                 cT[:, k], start=(k == 0), stop=False)
            nc.tensor.matmul(ps_st, ones[:1, :QL], cbias[:1, bi],
                             start=False, stop=True)
            nmax2 = small.tile([64, 1], f32, tag="nmax2")
            nc.vector.reduce_max(nmax2, ps_st, axis=mybir.AxisListType.X,
                                 negate=True)
            e2 = small.tile([64, CL], f32, tag="e2")
            sum2 = small.tile([64, 1], f32, tag="sum2")
            nc.scalar.activation(e2, ps_st, mybir.ActivationFunctionType.Exp,
                                 bias=nmax2, scale=1.0, accum_out=sum2)
            r2 = small.tile([64, 1], f32, tag="r2")
            nc.vector.reciprocal(r2, sum2)
            s2T = small.tile([64, CL], f32, tag="s2T")
            nc.vector.tensor_scalar_mul(s2T, e2, r2)
            # transpose back: s2[c, q]
            s2 = small.tile([128, CT, QL], f32, tag="s2")
            for ci in range(CT):
                tp = ptp.tile([128, QL], f32, tag="tp")
                nc.tensor.transpose(tp, s2T[:, ci * 128:(ci + 1) * 128],
                                    ident[:64, :64])
                nc.vector.tensor_copy(out=s2[:, ci, :], in_=tp)

            # --- s[c, q] per c-tile, row softmax -> s1, transpose -> s1T ---
            s1T = small.tile([64, CL], rdt, tag="s1T")
            for ci in range(CT):
                ps_s = pacc.tile([128, QL], f32, tag="acc")
                for k in range(HK):
                    nc.tensor.matmul(
                        ps_s, cT[:, k, ci * 128:(ci + 1) * 128],
                        qwt[:, k, bi * QL:(bi + 1) * QL],
                        start=(k == 0), stop=False)
                nc.tensor.matmul(ps_s, ones[:1, :128], qrow[:1, bi],
                                 start=False, stop=True)
                nmax1 = small.tile([128, 1], f32, tag="nmax1")
                nc.vector.reduce_max(nmax1, ps_s, axis=mybir.AxisListType.X,
                                     negate=True)
                e1 = small.tile([128, QL], f32, tag="e1")
                sum1 = small.tile([128, 1], f32, tag="sum1")
                nc.scalar.activation(e1, ps_s, mybir.ActivationFunctionType.Exp,
                                     bias=nmax1, scale=1.0, accum_out=sum1)
                r1 = small.tile([128, 1], f32, tag="r1")
                nc.vector.reciprocal(r1, sum1)
                s1 = small.tile([128, QL], f32, tag="s1")
                nc.vector.tensor_scalar_mul(s1, e1, r1)
                tp = ptp.tile([64, 128], f32, tag="tp")
                nc.tensor.transpose(tp, s1, ident)
                nc.scalar.copy(out=s1T[:, ci * 128:(ci + 1) * 128], in_=tp)

            # --- qc[q, h] = s2.T @ c ---
            qc = perb.tile([64, H], rdt, tag="qc")
            for hf in range(2):
                ps_qc = pacc.tile([64, NH], f32, tag="acc")
                for ci in range(CT):
                    nc.tensor.matmul(ps_qc, s2[:, ci, :],
                                     c_nat[:, ci, hf * NH:(hf + 1) * NH],
                                     start=(ci == 0), stop=(ci == CT - 1))
                nc.scalar.copy(out=qc[:, hf * NH:(hf + 1) * NH], in_=ps_qc)

            # --- a = s1 @ q ; bv = s1 @ qc ; outputs ---
            for ci in range(CT):
                rows = slice(ci * 128, (ci + 1) * 128)
                a_sb = outp.tile([128, H], f32, tag="a")
                ca_sb = outp.tile([128, H], f32, tag="ca")
                cbv_sb = outp.tile([128, H], f32, tag="cbv")
                for hf in range(2):
                    cols = slice(hf * NH, (hf + 1) * NH)
                    ps_a = pacc.tile([128, NH], f32, tag="acc")
                    nc.tensor.matmul(ps_a, s1T[:, rows], q_r[bi][:, cols],
                                     start=True, stop=True)
                    nc.scalar.copy(out=a_sb[:, cols], in_=ps_a)
                    nc.vector.tensor_mul(ca_sb[:, cols], c_nat[:, ci, cols],
                                         a_sb[:, cols])
                    ps_bv = pacc.tile([128, NH], f32, tag="acc")
                    nc.tensor.matmul(ps_bv, s1T[:, rows], qc[:, cols],
                                     start=True, stop=True)
                    nc.vector.tensor_mul(cbv_sb[:, cols], c_nat[:, ci, cols],
                                         ps_bv)
                nc.sync.dma_start(out=outD[bi, rows, 0:H], in_=c_nat[:, ci, :])
                nc.sync.dma_start(out=outD[bi, rows, H:2 * H], in_=a_sb)
                nc.sync.dma_start(out=outD[bi, rows, 2 * H:3 * H], in_=ca_sb)
                nc.sync.dma_start(out=outD[bi, rows, 3 * H:4 * H], in_=cbv_sb)

    nc.finalize()
    return nc


_NC_CACHE: dict = {}


def _get_nc(use_f32r: bool) -> bass.Bass:
    if use_f32r not in _NC_CACHE:
        _NC_CACHE[use_f32r] = _build_nc(use_f32r)
    return _NC_CACHE[use_f32r]


def kernel(c, q, c_mask, q_mask, W, b, _trace=False, _use_f32r=True):
    nc = _get_nc(_use_f32r)
    in_maps = []
    for i in range(NCORES):
        sl = slice(i * BPC, (i + 1) * BPC)
        in_maps.append({
            "c": np.ascontiguousarray(np.asarray(c)[sl], dtype=np.float32),
            "q": np.ascontiguousarray(np.asarray(q)[sl], dtype=np.float32),
            "c_mask": np.ascontiguousarray(np.asarray(c_mask)[sl], dtype=np.int32),
            "q_mask": np.ascontiguousarray(np.asarray(q_mask)[sl], dtype=np.int32),
            "W": np.ascontiguousarray(np.asarray(W), dtype=np.float32),
            "b": np.ascontiguousarray(np.asarray(b), dtype=np.float32),
        })
    res = run_bass_kernel_spmd(nc, in_maps, core_ids=list(range(NCORES)),
                               trace=_trace)
    out = np.concatenate([res.results[i]["out"] for i in range(NCORES)], axis=0)
    if _trace:
        return out, res
    return out
